# revision 8
# baseline (speedup 1.0000x reference)
"""DGCNN centerline feature-fusion kernel for 8 Trainium2 NeuronCores.

Problem: for each point n with 4 neighbor indices idx[n,k] and weights
w[n,k]:  out[:, n] = concat(pv[n,0:3], (pv[n,11] if boundary),
                            sum_k feature[idx[n,k]] / max(w[n,k],1e-10))
transposed to [C, N] channel-major layout.

Sharding: data-parallel over (batch b, N-half h) -> 8 shards; each core
handles both the "o" and "b" streams of its shard with its own per-batch
feature banks, so gathers stay local to the core.

Per-core dataflow (points-on-partitions):
  - pv rows DMA'd as [128, T, 12] tiles (point n = n0 + t*128 + p)
  - indices floor()ed to int16, folded into the SWDGE gather wrap layout
    ([16, ni/16] replicated to all 8 Q7 partition groups)
  - one 8192-index dma_gather per 2048-point group pulls feature rows from
    HBM into G[128, 64, 64] (slot (t,k) per partition)
  - DVE: G *= 1/max(w,1e-10) (free-dim broadcast), then pairwise k-adds
    into acc[128, 16, 68]
  - PE transposes acc tiles to PSUM [C, 128] (channel-major)
  - ACT copies PSUM -> SBUF out buffer; HWDGE writes [C, 2048] spans out
"""

import sys

for _p in ("/opt/trn_rl_repo", "/root/.axon_site/_ro/trn_rl_repo"):
    if _p not in sys.path:
        sys.path.append(_p)

import numpy as np

import concourse.bacc as bacc
import concourse.mybir as mybir
import concourse.tile as tile_mod
from concourse.masks import make_identity
from concourse.tile import TileContext
from concourse.vector_clock import ScopedClock

# ---------------------------------------------------------------- tile patch
# This walrus build allows only one sync-wait command per TPB_CTRL
# instruction; the Tile exit drain carries one wait per clock domain. Split
# the excess onto NOPs placed just before the drain (same engine, program
# order => same semantics).
_MAX_WAITS = 1


def _split_waits(nc, inst):
    si = inst.sync_info
    waits = list(si.on_wait) if si and si.on_wait else []
    if len(waits) <= _MAX_WAITS:
        return
    keep = waits[-_MAX_WAITS:]
    extra = waits[:-_MAX_WAITS]
    si.on_wait = keep
    bb = nc.cur_bb.bb
    insts = bb.instructions
    pos = insts.index(inst)
    nops = []
    for i in range(0, len(extra), _MAX_WAITS):
        nop = nc.engines[inst.engine].nop(nofuse=True).ins
        nop.sync_info = mybir.SyncInfo(
            on_wait=extra[i : i + _MAX_WAITS], on_update=[]
        )
        nops.append(nop)
    for nop in nops:
        insts.remove(nop)
    for j, nop in enumerate(nops):
        insts.insert(pos + j, nop)


def _patched_drain_and_barrier(self, tick_clock, wait_clock):
    nc = self.nc
    drain_inst = nc.sync.drain()
    wait_clock.add_sem_waits(
        drain_inst.ins, ScopedClock({None: tick_clock.global_clock})
    )
    _split_waits(nc, drain_inst.ins)
    nc.all_engine_barrier()
    assert self.sems is not None
    popped = nc._tile_sem_poison_stack.pop()
    assert popped is self._sem_poison
    nc.clear_and_free_semaphores(list(self.sems.allocated().values()))
    nc.all_engine_barrier()


tile_mod.TileContext._drain_and_barrier = _patched_drain_and_barrier

# ---------------------------------------------------------------- constants
P = 128
D = 64
F = 2048
M = 12
N_SH = 32768  # points per core per stream (65536 N-half... 32768)
GG_PTS = 2048  # points per gather group (one dma_gather of 8192 idxs)
IG_PTS = 4096  # points per index-prep group (2 gather groups)
NI = GG_PTS * 4  # 8192 indices per gather
N_QUEUES = 4
MIN_W = 1e-10

f32 = mybir.dt.float32
i16 = mybir.dt.int16


def _build_core_kernel(reps=1):
    """Build the per-core Bacc program (identical for all 8 cores).

    reps > 1 repeats the whole workload on-device (for marginal-cost
    timing in test harnesses); outputs are simply overwritten.
    """
    nc = bacc.Bacc(
        "TRN2", target_bir_lowering=False, debug=False, num_swdge_queues=N_QUEUES
    )

    pv = {
        "o": nc.dram_tensor("pv_o", [N_SH, M], f32, kind="ExternalInput"),
        "b": nc.dram_tensor("pv_b", [N_SH, M], f32, kind="ExternalInput"),
    }
    feat = {
        "o": nc.dram_tensor("feat_o", [F, D], f32, kind="ExternalInput"),
        "b": nc.dram_tensor("feat_b", [F, D], f32, kind="ExternalInput"),
    }
    outs = {
        "o": nc.dram_tensor("out_o", [3 + D, N_SH], f32, kind="ExternalOutput"),
        "b": nc.dram_tensor("out_b", [4 + D, N_SH], f32, kind="ExternalOutput"),
    }
    n_ch = {"o": 3 + D, "b": 4 + D}  # 67 / 68
    ch0 = {"o": 3, "b": 4}  # first weighted channel

    qc = 0  # rotating SWDGE queue counter

    with TileContext(nc) as tc:
        with (
            tc.tile_pool(name="const", bufs=1) as const_pool,
            tc.tile_pool(name="pv", bufs=2) as pv_pool,
            tc.tile_pool(name="w", bufs=2) as w_pool,
            tc.tile_pool(name="idx", bufs=2) as idx_pool,
            tc.tile_pool(name="g", bufs=3) as g_pool,
            tc.tile_pool(name="tmp", bufs=2) as tmp_pool,
            tc.tile_pool(name="acc", bufs=3) as acc_pool,
            tc.tile_pool(name="ob", bufs=2) as ob_pool,
            tc.tile_pool(name="ps", bufs=4, space="PSUM") as ps_pool,
            tc.tile_pool(name="psx", bufs=1, space="PSUM") as psx_pool,
            tc.tile_pool(name="pso", bufs=2, space="PSUM") as pso_pool,
        ):
            ident = const_pool.tile([P, P], f32)
            make_identity(nc, ident[:])

            T_IG = IG_PTS // P  # 32 t-blocks per index group
            T_GG = GG_PTS // P  # 16 t-blocks per gather group
            n_ig = N_SH // IG_PTS  # 8 per stream

            for rep_ig in range(reps * 2 * n_ig):
                ig = rep_ig % (2 * n_ig)
                s = "o" if ig % 2 == 0 else "b"
                grp = ig // 2
                n0 = grp * IG_PTS

                # ---- load pv rows: [128, 32, 12], point n0 + t*128 + p
                pv_sb = pv_pool.tile([P, T_IG, M], f32)
                nc.sync.dma_start(
                    out=pv_sb[:],
                    in_=pv[s][n0 : n0 + IG_PTS].rearrange(
                        "(t p) c -> p t c", p=P
                    ),
                )

                # ---- weights: winv = 1/max(w, MIN_W)   [128, 128]
                wmax = w_pool.tile([P, T_IG * 4], f32, tag="wmax")
                nc.vector.tensor_scalar_max(
                    wmax[:].rearrange("p (t k) -> p t k", k=4),
                    pv_sb[:, :, 7:11],
                    MIN_W,
                )
                winv = w_pool.tile([P, T_IG * 4], f32, tag="winv")
                nc.vector.reciprocal(winv[:], wmax[:])

                # ---- indices: floor(pv[3:7]) kept in f32 (exact ints).
                # The HW f32->int convert rounds to nearest-even, so fix up:
                # xb = rne(x); floor = xb - (xb > x).
                xsl = pv_sb[:, :, 3:7]
                ii = idx_pool.tile([P, T_IG * 4], i16, tag="ii")
                nc.vector.tensor_copy(
                    ii[:].rearrange("p (t k) -> p t k", k=4), xsl
                )
                xb = w_pool.tile([P, T_IG * 4], f32, tag="xb")
                nc.vector.tensor_copy(xb[:], ii[:])
                fixf = w_pool.tile([P, T_IG * 4], f32, tag="fixf")
                nc.vector.tensor_tensor(
                    fixf[:].rearrange("p (t k) -> p t k", k=4),
                    xb[:].rearrange("p (t k) -> p t k", k=4),
                    xsl,
                    op=mybir.AluOpType.is_gt,
                )
                idxf = idx_pool.tile([P, T_IG * 4], f32, tag="idxf")
                nc.vector.tensor_tensor(
                    idxf[:], xb[:], fixf[:], op=mybir.AluOpType.subtract
                )

                # ---- fold to the SWDGE wrap layout idxw[p%16, c*8 + p//16].
                # DVE can't address 16-granular partition starts, so go
                # through PE: transpose to [c, p], then per 16-partition
                # group g a narrow transpose-back lands that group's
                # indices at partitions 0:16.
                C_IG = T_IG * 4  # 128 slots
                p1psum = psx_pool.tile([P, C_IG], f32, tag="idxT")
                nc.tensor.transpose(p1psum[:], idxf[:], ident[:])
                p1s = idx_pool.tile([P, C_IG], f32, tag="p1s")
                nc.scalar.copy(p1s[:], p1psum[:])
                idxw = idx_pool.tile([16 * 8, C_IG * 8], i16, tag="idxw")
                idxw_v = idxw[0:16, :].rearrange("q (c g) -> q c g", g=8)
                for g in range(8):
                    og = pso_pool.tile([16, C_IG], f32, tag="og")
                    nc.tensor.transpose(
                        og[:], p1s[:, g * 16 : (g + 1) * 16], ident[:]
                    )
                    nc.scalar.copy(idxw_v[:, :, g], og[:])
                # replicate [16] -> all 8 Q7 partition groups (log2 doubling)
                nc.sync.dma_start(out=idxw[16:32, :], in_=idxw[0:16, :])
                nc.sync.dma_start(out=idxw[32:64, :], in_=idxw[0:32, :])
                nc.sync.dma_start(out=idxw[64:128, :], in_=idxw[0:64, :])

                for gg in range(2):
                    gn0 = n0 + gg * GG_PTS
                    # ---- gather 8192 feature rows for this 2048-pt group
                    G = g_pool.tile([P, NI // P, D], f32)
                    nc.gpsimd.dma_gather(
                        G[:],
                        feat[s][:],
                        idxw[:, gg * (NI // 16) : (gg + 1) * (NI // 16)],
                        NI,
                        NI,
                        D,
                        single_packet=False,
                        queue_num=qc % N_QUEUES,
                    )
                    qc += 1

                    # ---- weighted sum over k
                    g4 = G[:].rearrange("p (t k) d -> p t k d", k=4)
                    wslice = winv[:, gg * (T_GG * 4) : (gg + 1) * (T_GG * 4)]
                    nc.vector.tensor_tensor(
                        G[:],
                        G[:],
                        wslice[:, :, None].to_broadcast((P, T_GG * 4, D)),
                        op=mybir.AluOpType.mult,
                    )
                    tA = tmp_pool.tile([P, T_GG, D], f32, tag="tA")
                    tB = tmp_pool.tile([P, T_GG, D], f32, tag="tB")
                    nc.vector.tensor_add(tA[:], g4[:, :, 0, :], g4[:, :, 1, :])
                    nc.vector.tensor_add(tB[:], g4[:, :, 2, :], g4[:, :, 3, :])

                    acc = acc_pool.tile([P, T_GG, 68], f32)
                    nc.vector.tensor_add(
                        acc[:, :, ch0[s] : ch0[s] + D], tA[:], tB[:]
                    )

                    # ---- passthrough channels
                    tsl = slice(gg * T_GG, (gg + 1) * T_GG)
                    nc.scalar.copy(acc[:, :, 0:3], pv_sb[:, tsl, 0:3])
                    if s == "b":
                        nc.scalar.copy(acc[:, :, 3:4], pv_sb[:, tsl, 11:12])

                    # ---- transpose to channel-major + store
                    C = n_ch[s]
                    obuf = ob_pool.tile([P, GG_PTS], f32)
                    for q in range(4):
                        pst = ps_pool.tile([P, 512], f32)
                        for t4 in range(4):
                            t = q * 4 + t4
                            nc.tensor.transpose(
                                pst[:C, t4 * P : (t4 + 1) * P],
                                acc[:, t, 0:C],
                                ident[:],
                            )
                        nc.scalar.copy(
                            obuf[:C, q * 512 : (q + 1) * 512], pst[:C, :]
                        )
                    nc.sync.dma_start(
                        out=outs[s][:, gn0 : gn0 + GG_PTS], in_=obuf[:C, :]
                    )

    nc.compile()
    return nc


_NC_CACHE = None


def _get_nc():
    global _NC_CACHE
    if _NC_CACHE is None:
        _NC_CACHE = _build_core_kernel()
    return _NC_CACHE


def _shard_inputs(pv_o, pv_b, feature_o, feature_b):
    n_half = pv_o.shape[1] // 2
    assert n_half == N_SH
    in_maps = []
    for c in range(8):
        b, h = c // 2, c % 2
        sl = slice(h * n_half, (h + 1) * n_half)
        in_maps.append(
            {
                "pv_o": np.ascontiguousarray(pv_o[b, sl]),
                "pv_b": np.ascontiguousarray(pv_b[b, sl]),
                "feat_o": np.ascontiguousarray(feature_o[b]),
                "feat_b": np.ascontiguousarray(feature_b[b]),
            }
        )
    return in_maps


def kernel(pv_o, pv_b, feature_o, feature_b):
    from concourse.bass_utils import run_bass_kernel_spmd

    B, N, _ = pv_o.shape  # [4, 65536, 12]
    n_half = N // 2  # 32768

    nc = _get_nc()
    in_maps = _shard_inputs(pv_o, pv_b, feature_o, feature_b)
    res = run_bass_kernel_spmd(nc, in_maps, core_ids=list(range(8)))

    out_o = np.empty((B, 3 + D, N), dtype=np.float32)
    out_b = np.empty((B, 4 + D, N), dtype=np.float32)
    for c in range(8):
        b, h = c // 2, c % 2
        sl = slice(h * n_half, (h + 1) * n_half)
        out_o[b, :, sl] = res.results[c]["out_o"]
        out_b[b, :, sl] = res.results[c]["out_b"]
    return out_o, out_b
